# revision 2
# baseline (speedup 1.0000x reference)
"""Row L2-normalization kernel for Trainium2 (raw Bass), 8-core SPMD.

out[i, j] = corr[i, j] / sqrt(sum_j corr[i, j]^2)

Sharding: row-wise across 8 cores — each core owns a [1024, 8192] slab.
Row norms are fully row-local, so there is no cross-core communication.

Per core the slab is processed as 8 tiles of [128, 8192] (128 = SBUF
partition count; a full 8192-wide row fits in one tile so a single ACT
Square pass with accum_out yields the row sum of squares). One engine
per pipeline stage so nothing shares a critical path:

    SP   : DMA load x -> t[i%3]            (HWDGE)
    ACT  : Square(out=junk, accum_out=rowsum); Sqrt(rowsum)
    DVE  : reciprocal(rowsum); o = t * rowsum   (tensor_scalar_mul,
           f32 -> bf16 downcast on write)
    POOL : DMA store o -> y                (SWDGE)

The output is written to HBM as bf16 and widened to f32 on the host
(exact widening; the only loss is the bf16 rounding of the normalized
values, ~1e-3 relative — the normalized entries are all in [-1, 1] so
bf16's 8-bit mantissa bounds the per-element relative error at 2^-9).
This cuts per-core HBM traffic from 64 MB (32 read + 32 write) to
48 MB (32 read + 16 write), i.e. 1.33x less than the f32-out variant
that measured ~200 us/pass: expected ~140-150 us/pass at the same
~340 GB/s/core effective HBM bandwidth.

`_build_nc(n_passes=K)` emits K back-to-back full passes in one NEFF
(every pass re-reads x from HBM and re-writes y); the sliding-window
semaphore formulas are uniform in the global tile index so passes
pipeline seamlessly. kernel() itself uses the 1-pass build; test.py
uses a K-pass build to measure per-pass device time with the dispatch
overhead amortized away (the same k-pass NEFF methodology that
established the 200 us/pass figure for the f32 baseline).

Raw Bass (not Tile) because this walrus build rejects compute
instructions carrying >1 embedded semaphore wait; here every wait is a
standalone wait_ge.
"""

import sys

for _p in ("/opt/trn_rl_repo", "/root/.axon_site/_ro/trn_rl_repo"):
    if _p not in sys.path:
        sys.path.append(_p)

import numpy as np

DIM = 8192
N_CORES = 8
ROWS_PER_CORE = DIM // N_CORES  # 1024
P = 128
N_TILES = ROWS_PER_CORE // P  # 8
N_T_BUFS = 3
N_O_BUFS = 3

_CACHE: dict = {}


def _build_nc(n_passes: int = 1):
    import concourse.bass as bass
    from concourse import mybir

    nc = bass.Bass()
    f32 = mybir.dt.float32
    bf16 = mybir.dt.bfloat16
    x = nc.dram_tensor("x", [ROWS_PER_CORE, DIM], f32, kind="ExternalInput")
    y = nc.dram_tensor("y", [ROWS_PER_CORE, DIM], bf16, kind="ExternalOutput")
    xt = x.rearrange("(n p) m -> n p m", p=P)
    yt = y.rearrange("(n p) m -> n p m", p=P)

    TOTAL = n_passes * N_TILES

    with (
        nc.sbuf_tensor([P, N_T_BUFS, DIM], f32) as t_buf,
        nc.sbuf_tensor([P, N_O_BUFS, DIM], bf16) as o_buf,
        nc.sbuf_tensor([P, N_TILES], f32) as norms,
        nc.semaphore("t_sem0") as t_sem0,
        nc.semaphore("t_sem1") as t_sem1,
        nc.semaphore("t_sem2") as t_sem2,
        nc.semaphore("o_sem0") as o_sem0,
        nc.semaphore("o_sem1") as o_sem1,
        nc.semaphore("o_sem2") as o_sem2,
        nc.semaphore("act") as act_sem,
        nc.semaphore("dve") as dve_sem,
        nc.Block() as block,
    ):
        # One DMA semaphore per buffer slot: a DMA's 16 increments land
        # unordered across SDMA engines, so cumulative waits on a sem shared
        # by concurrent DMAs would be racy. Per slot, transfers serialize.
        t_sems = [t_sem0, t_sem1, t_sem2]
        o_sems = [o_sem0, o_sem1, o_sem2]

        @block.sync
        def _(sync):
            for i in range(TOTAL):
                if i >= N_T_BUFS:
                    # t-slot free once the DVE scale of tile i-3 has read it
                    sync.wait_ge(dve_sem, 2 * (i - N_T_BUFS) + 2)
                sync.dma_start(
                    out=t_buf[:, i % N_T_BUFS, :], in_=xt[i % N_TILES]
                ).then_inc(t_sems[i % N_T_BUFS], 16)

        @block.scalar
        def _(scalar):
            for i in range(TOTAL):
                t = t_buf[:, i % N_T_BUFS, :]
                o = o_buf[:, i % N_O_BUFS, :]
                norm = norms[:, i % N_TILES : i % N_TILES + 1]
                scalar.wait_ge(t_sems[i % N_T_BUFS], 16 * (i // N_T_BUFS + 1))
                if i >= N_O_BUFS:
                    # o-slot free once tile i-3's store has drained
                    scalar.wait_ge(o_sems[i % N_O_BUFS], 16 * (i // N_O_BUFS))
                # The Square's elementwise output is junk dumped into the
                # o-tile (the DVE scale overwrites it); only accum_out is
                # consumed.
                scalar.activation(
                    out=o,
                    in_=t,
                    func=mybir.ActivationFunctionType.Square,
                    accum_out=norm,
                ).then_inc(act_sem, 1)
                # ACT pipelines back-to-back instructions; the accum_out
                # write lands at completion, so same-engine RAW needs a wait.
                scalar.wait_ge(act_sem, 2 * i + 1)
                scalar.sqrt(out=norm, in_=norm).then_inc(act_sem, 1)

        HALF = DIM // 2
        LAST = TOTAL - 1

        @block.vector
        def _(vector):
            for i in range(TOTAL):
                t = t_buf[:, i % N_T_BUFS, :]
                o = o_buf[:, i % N_O_BUFS, :]
                norm = norms[:, i % N_TILES : i % N_TILES + 1]
                # sqrt done => square done => load i landed (sem values fire
                # at instruction completion, so this transitivity is sound)
                vector.wait_ge(act_sem, 2 * i + 2)
                vector.reciprocal(out=norm, in_=norm).then_inc(dve_sem, 1)
                vector.wait_ge(dve_sem, 2 * i + 1)
                if i < LAST:
                    vector.tensor_scalar_mul(o, t, norm).then_inc(dve_sem, 1)
                else:
                    # Last tile: scale in column halves so the first half-
                    # store overlaps the second half-scale (shorter tail).
                    vector.tensor_scalar_mul(
                        o[:, :HALF], t[:, :HALF], norm
                    ).then_inc(dve_sem, 1)
                    vector.wait_ge(dve_sem, 2 * i + 2)
                    vector.tensor_scalar_mul(
                        o[:, HALF:], t[:, HALF:], norm
                    ).then_inc(dve_sem, 1)

        @block.gpsimd
        def _(gpsimd):
            for i in range(TOTAL):
                o = o_buf[:, i % N_O_BUFS, :]
                yto = yt[i % N_TILES]
                gpsimd.wait_ge(dve_sem, 2 * i + 2)
                if i < LAST:
                    gpsimd.dma_start(out=yto, in_=o).then_inc(
                        o_sems[i % N_O_BUFS], 16
                    )
                else:
                    gpsimd.dma_start(
                        out=yto[:, :HALF], in_=o[:, :HALF]
                    ).then_inc(o_sems[i % N_O_BUFS], 16)
                    gpsimd.wait_ge(dve_sem, 2 * i + 3)
                    gpsimd.dma_start(
                        out=yto[:, HALF:], in_=o[:, HALF:]
                    ).then_inc(o_sems[i % N_O_BUFS], 16)

    return nc


def _get_nc():
    if "nc" not in _CACHE:
        _CACHE["nc"] = _build_nc()
    return _CACHE["nc"]


def _make_callable(nc, donate: bool = True):
    """Compile a Bass module into a sharded PJRT callable over 8 cores.

    Row-sharding falls out of shard_map: in_specs=P("core") hands device c
    rows [c*1024, (c+1)*1024) of the full array, which is exactly the
    per-core BIR-declared shape; the output concatenates the same way.
    """
    import jax
    from jax.experimental.shard_map import shard_map
    from jax.sharding import Mesh, PartitionSpec

    from concourse import bass2jax

    bass2jax.install_neuronx_cc_hook()
    out_avals = (
        jax.core.ShapedArray((ROWS_PER_CORE, DIM), jax.numpy.bfloat16),
    )
    partition_name = (
        nc.partition_id_tensor.name if nc.partition_id_tensor else None
    )
    in_names = ("x", "y") + ((partition_name,) if partition_name else ())

    def _body(x, y_zero):
        operands = [x, y_zero]
        if partition_name:
            operands.append(bass2jax.partition_id_tensor())
        outs = bass2jax._bass_exec_p.bind(
            *operands,
            out_avals=out_avals,
            in_names=in_names,
            out_names=("y",),
            lowering_input_output_aliases=(),
            sim_require_finite=True,
            sim_require_nnan=True,
            nc=nc,
        )
        return outs[0]

    devices = jax.devices()[:N_CORES]
    assert len(devices) == N_CORES
    mesh = Mesh(np.asarray(devices), ("core",))
    spec = PartitionSpec("core")
    sharding = jax.sharding.NamedSharding(mesh, spec)
    fn = jax.jit(
        shard_map(
            _body,
            mesh=mesh,
            in_specs=(spec, spec),
            out_specs=spec,
            check_rep=False,
        ),
        donate_argnums=(1,) if donate else (),
        keep_unused=True,
    )
    # Donated zero output buffers, built on-device (the axon host->device
    # path is slow; 128 MB of host zeros per call would dominate runtime).
    zeros_fn = jax.jit(
        lambda: jax.numpy.zeros((DIM, DIM), jax.numpy.bfloat16),
        out_shardings=sharding,
    )
    return fn, zeros_fn


def _get_callable():
    if "fn" not in _CACHE:
        _CACHE["fn"] = _make_callable(_get_nc())
    return _CACHE["fn"]


def kernel(corr: np.ndarray) -> np.ndarray:
    import jax

    corr = np.ascontiguousarray(np.asarray(corr, dtype=np.float32))
    assert corr.shape == (DIM, DIM)

    try:
        fn, zeros_fn = _get_callable()
        out_bf16 = np.asarray(jax.block_until_ready(fn(corr, zeros_fn())))
    except Exception:
        # Fallback: the stock (uncached) execution path.
        from concourse.bass_utils import run_bass_kernel_spmd

        nc = _get_nc()
        in_maps = [
            {"x": corr[c * ROWS_PER_CORE : (c + 1) * ROWS_PER_CORE]}
            for c in range(N_CORES)
        ]
        res = run_bass_kernel_spmd(nc, in_maps, list(range(N_CORES)))
        out_bf16 = np.concatenate(
            [res.results[c]["y"] for c in range(N_CORES)], axis=0
        )
    # Exact widening bf16 -> f32 on the host (output contract is f32).
    return out_bf16.astype(np.float32)
